# revision 3
# baseline (speedup 1.0000x reference)
"""Trainium2 Bass kernel for GCNGraphClassifier.

Model (see reference): 2-layer GCN (sym-normalized, self-loops) over one big
random graph of N=50000 nodes / E=600000 edges, global mean-pool by graph id
(G=128 graphs), MLP head (Linear-ReLU-LayerNorm-Linear) -> [G, 16] logits.

Sharding: nodes/edges are graph-parallel across 8 cores (16 graphs per core,
batch_idx is sorted so each core owns a contiguous node range). Each core owns
the edges whose *destination* lands in its range and produces its node-shard
of each conv layer. Weights are replicated. A mid-kernel AllGather exchanges
the layer-1 activations (each core needs arbitrary source rows for layer 2).
Final [16,16] logit shards are concatenated on host.

Device algorithm (aggregate-first GCN, which commutes with the linear layer):
  aggT[d, i] = sum_e norm_e * x[src_e, d] * (dst_e == i)      per 128-node window
computed as a PE matmul  aggT += G_chunk.T @ S_chunk  where G_chunk is a
dma_gather of 128 source rows and S_chunk = (iota == dst_local) * norm is one
fused DVE tensor_scalar op. The window result then hits the weight matmul
(out = aggT.T @ W + b), bias is pre-loaded into PSUM, ReLU fused in the
ScalarE eviction. Pooling folds 1/count into the same selector-matmul trick.
"""

import sys

sys.path.insert(0, "/opt/trn_rl_repo")

import numpy as np

CORES = 8
N = 50000
E = 600000
D = 128
H = 128
CO = 16
G = 128
GPC = G // CORES  # graphs per core
P = 128
T1 = 25088  # conv1 gather table split (both halves < 32768 for int16 idx)


def _wrap_idx16(idx_vals, j, base8, width):
    """Scatter gather-indices into the Q7 wrapped layout: within one
    dma_gather call, slot j -> partition j%16, column j//16. base8 is the
    per-section absolute column start (8*chunk_base)."""
    out = np.zeros((16, width), np.int16)
    out[j % 16, base8 + j // 16] = idx_vals
    return out


def _build_conv_meta(src_idx, half_flag, seg_id, order, edlocal, norm_all,
                     edge_core, n_seg_per_core, WPC):
    """Build per-core gather-index / dst-local / norm arrays for one conv.

    src_idx: gather row within its half-table, per edge (int64)
    half_flag: 0/1 per edge
    seg_id: (core*WPC + window)*2 + half  per edge
    order: stable argsort of seg_id
    Returns (C (chunks per [w,half]), i16 list, d list, n list) per core.
    """
    Etot = len(src_idx)
    counts = np.bincount(seg_id, minlength=CORES * n_seg_per_core)
    # uniform chunk structure across cores: max count per (window, half)
    cmax = counts.reshape(CORES, WPC, 2).max(axis=0)
    C = np.ceil(cmax / P).astype(np.int64)  # [WPC, 2] chunks
    base = np.concatenate([[0], np.cumsum(C.reshape(-1))])[:-1].reshape(WPC, 2)
    TC = int(C.sum())

    seg_sorted = seg_id[order]
    seg_starts = np.concatenate([[0], np.cumsum(counts)])
    j_within = np.arange(Etot, dtype=np.int64) - seg_starts[seg_sorted]

    wh = seg_sorted % n_seg_per_core
    w_e = wh // 2
    h_e = wh % 2
    base_e = base[w_e, h_e]  # chunk base of each edge's section
    core_e = seg_sorted // n_seg_per_core

    src_s = src_idx[order]
    dl_s = edlocal[order]
    nm_s = norm_all[order]

    i16s, ds, ns = [], [], []
    for k in range(CORES):
        m = core_e == k
        jm = j_within[m]
        i16 = _wrap_idx16(src_s[m], jm, base_e[m] * 8, 8 * TC)
        d = np.zeros((P, TC), np.float32)
        n = np.zeros((P, TC), np.float32)
        col = base_e[m] + jm // P
        row = jm % P
        d[row, col] = dl_s[m]
        n[row, col] = nm_s[m]
        i16s.append(np.tile(i16, (8, 1)))
        ds.append(d)
        ns.append(n)
    return C, base, TC, i16s, ds, ns


def _host_prep(x, edge_index, edge_weight, batch_idx):
    x = np.asarray(x, np.float32)
    ei = np.asarray(edge_index, np.int64)
    ew = np.asarray(edge_weight, np.float32)
    bi = np.asarray(batch_idx, np.int64)

    loops = np.arange(N, dtype=np.int64)
    src_all = np.concatenate([ei[0], loops])
    dst_all = np.concatenate([ei[1], loops])
    ew_all = np.concatenate([ew, np.ones(N, np.float32)]).astype(np.float32)

    deg = np.bincount(dst_all, weights=ew_all.astype(np.float64),
                      minlength=N).astype(np.float32)
    dinv = np.where(deg > 0, 1.0 / np.sqrt(deg), 0.0).astype(np.float32)
    norm_all = (dinv[src_all] * ew_all * dinv[dst_all]).astype(np.float32)

    # node shard boundaries at graph boundaries (batch_idx is sorted)
    gstart = np.searchsorted(bi, np.arange(0, G + 1, GPC)).astype(np.int64)
    nk = np.diff(gstart)
    NMAX = int(np.ceil(nk.max() / P) * P)
    WPC = NMAX // P
    assert CORES * NMAX <= 65536, "padded table too large for int16 gather"
    T2 = (CORES * NMAX) // 2

    node_core = bi // GPC
    local_id = np.arange(N, dtype=np.int64) - gstart[node_core]
    pgid = node_core * NMAX + local_id  # padded global row (conv2 table)

    edge_core = node_core[dst_all]
    eloc = local_id[dst_all]
    ewin = eloc // P
    edlocal = (eloc % P).astype(np.float32)
    n_seg = WPC * 2

    # conv1: split by raw src row
    h1f = (src_all >= T1).astype(np.int64)
    seg1 = (edge_core * WPC + ewin) * 2 + h1f
    o1 = np.argsort(seg1, kind="stable")
    src1 = src_all - h1f * T1
    C1, base1, TC1, i1s, d1s, n1s = _build_conv_meta(
        src1, h1f, seg1, o1, edlocal, norm_all, edge_core, n_seg, WPC)

    # conv2: split by padded source row
    psrc = pgid[src_all]
    h2f = (psrc >= T2).astype(np.int64)
    seg2 = (edge_core * WPC + ewin) * 2 + h2f
    o2 = np.argsort(seg2, kind="stable")
    src2 = psrc - h2f * T2
    C2, base2, TC2, i2s, d2s, n2s = _build_conv_meta(
        src2, h2f, seg2, o2, edlocal, norm_all, edge_core, n_seg, WPC)

    # pooling metadata per core
    cnt = np.bincount(bi, minlength=G).astype(np.float32)
    invcnt = (1.0 / np.maximum(cnt, 1.0)).astype(np.float32)
    bls, ics = [], []
    for k in range(CORES):
        bl = np.full((P, WPC), 99.0, np.float32)
        ic = np.zeros((P, WPC), np.float32)
        nodes = np.arange(gstart[k], gstart[k + 1], dtype=np.int64)
        lid = local_id[nodes]
        bl[lid % P, lid // P] = (bi[nodes] - k * GPC).astype(np.float32)
        ic[lid % P, lid // P] = invcnt[bi[nodes]]
        bls.append(bl)
        ics.append(ic)

    return dict(x=x, NMAX=NMAX, WPC=WPC, T2=T2,
                C1=C1, base1=base1, TC1=TC1, i1s=i1s, d1s=d1s, n1s=n1s,
                C2=C2, base2=base2, TC2=TC2, i2s=i2s, d2s=d2s, n2s=n2s,
                bls=bls, ics=ics)


def _build_program(pp, want_trace=False):
    import concourse.bacc as bacc
    import concourse.tile as tile
    import concourse.mybir as mybir

    f32 = mybir.dt.float32
    i16t = mybir.dt.int16
    Alu = mybir.AluOpType
    Act = mybir.ActivationFunctionType

    WPC, NMAX, T2 = pp["WPC"], pp["NMAX"], pp["T2"]
    C1, TC1 = pp["C1"], pp["TC1"]
    C2, TC2 = pp["C2"], pp["TC2"]

    nc = bacc.Bacc("TRN2", target_bir_lowering=False, debug=False,
                   num_devices=CORES)

    xlo = nc.dram_tensor("xlo", [T1, D], f32, kind="ExternalInput")
    xhi = nc.dram_tensor("xhi", [N - T1, D], f32, kind="ExternalInput")
    i1 = nc.dram_tensor("i1", [P, 8 * TC1], i16t, kind="ExternalInput")
    d1 = nc.dram_tensor("d1", [P, TC1], f32, kind="ExternalInput")
    n1 = nc.dram_tensor("n1", [P, TC1], f32, kind="ExternalInput")
    i2 = nc.dram_tensor("i2", [P, 8 * TC2], i16t, kind="ExternalInput")
    d2 = nc.dram_tensor("d2", [P, TC2], f32, kind="ExternalInput")
    n2 = nc.dram_tensor("n2", [P, TC2], f32, kind="ExternalInput")
    blT = nc.dram_tensor("bl", [P, WPC], f32, kind="ExternalInput")
    icT = nc.dram_tensor("ic", [P, WPC], f32, kind="ExternalInput")
    W1T = nc.dram_tensor("W1", [D, H], f32, kind="ExternalInput")
    W2T = nc.dram_tensor("W2", [H, H], f32, kind="ExternalInput")
    l1w = nc.dram_tensor("l1w", [H, H], f32, kind="ExternalInput")
    l2w = nc.dram_tensor("l2w", [H, CO], f32, kind="ExternalInput")
    b1b = nc.dram_tensor("b1b", [P, H], f32, kind="ExternalInput")
    b2b = nc.dram_tensor("b2b", [P, H], f32, kind="ExternalInput")
    l1b = nc.dram_tensor("l1b", [GPC, H], f32, kind="ExternalInput")
    l2b = nc.dram_tensor("l2b", [GPC, CO], f32, kind="ExternalInput")
    gamT = nc.dram_tensor("gam", [GPC, H], f32, kind="ExternalInput")
    betT = nc.dram_tensor("bet", [GPC, H], f32, kind="ExternalInput")
    iotaT = nc.dram_tensor("iota", [P, P], f32, kind="ExternalInput")
    iot16 = nc.dram_tensor("iot16", [P, GPC], f32, kind="ExternalInput")
    id16 = nc.dram_tensor("id16", [GPC, GPC], f32, kind="ExternalInput")
    logitsT = nc.dram_tensor("logits", [GPC, CO], f32, kind="ExternalOutput")

    CMAX1 = int(max(C1.max(), 1))
    CMAX2 = int(max(C2.max(), 1))

    with tile.TileContext(nc) as tc:
        with (
            tc.tile_pool(name="const", bufs=1) as cpool,
            tc.tile_pool(name="meta", bufs=1) as mpool,
            tc.tile_pool(name="gat", bufs=3) as gpool,
            tc.tile_pool(name="s", bufs=4) as spool,
            tc.tile_pool(name="work", bufs=3) as wpool,
            tc.tile_pool(name="psA", bufs=2, space="PSUM") as psA,
            tc.tile_pool(name="psB", bufs=2, space="PSUM") as psB,
            tc.tile_pool(name="psP", bufs=1, space="PSUM") as psP,
            tc.tile_pool(name="psH", bufs=2, space="PSUM") as psH,
            tc.tile_pool(name="dram", bufs=1, space="DRAM") as dpool,
        ):
            def load_const(t, src):
                tt = cpool.tile(list(src.shape), src.dtype, tag=src.name)
                nc.sync.dma_start(out=tt[:], in_=src[:])
                return tt

            iota_t = load_const(iotaT, iotaT)
            io16_t = load_const(iot16, iot16)
            W1_t = load_const(W1T, W1T)
            W2_t = load_const(W2T, W2T)
            b1_t = load_const(b1b, b1b)
            b2_t = load_const(b2b, b2b)
            bl_t = load_const(blT, blT)
            ic_t = load_const(icT, icT)

            i1_t = mpool.tile([P, 8 * TC1], i16t, tag="i1")
            d1_t = mpool.tile([P, TC1], f32, tag="d1")
            n1_t = mpool.tile([P, TC1], f32, tag="n1")
            i2_t = mpool.tile([P, 8 * TC2], i16t, tag="i2")
            d2_t = mpool.tile([P, TC2], f32, tag="d2")
            n2_t = mpool.tile([P, TC2], f32, tag="n2")
            for t, s in ((i1_t, i1), (d1_t, d1), (n1_t, n1),
                         (i2_t, i2), (d2_t, d2), (n2_t, n2)):
                nc.sync.dma_start(out=t[:], in_=s[:])

            h1_shard = dpool.tile([NMAX, H], f32, tag="h1s")
            h1_full = dpool.tile([CORES * NMAX, H], f32, tag="h1f",
                                 addr_space="Shared")

            def conv(C, base, i_t, d_t, n_t, tab_lo, tab_hi, W_t, bias_t,
                     relu, cmax, out_cb):
                for w in range(WPC):
                    gts = []
                    for hh, tab in ((0, tab_lo), (1, tab_hi)):
                        cwh = int(C[w, hh])
                        if cwh == 0:
                            gts.append(None)
                            continue
                        gt = gpool.tile([P, cmax, H], f32, tag=f"g{hh}")
                        off = int(base[w, hh])
                        nidx = cwh * P
                        nc.gpsimd.dma_gather(
                            gt[:, :cwh, :], tab[:], i_t[:, 8 * off:8 * (off + cwh)],
                            nidx, nidx, H, single_packet=False)
                        gts.append(gt)
                    aggT = psA.tile([P, P], f32, space="PSUM", tag="agg")
                    tot = int(C[w, 0] + C[w, 1])
                    cc = 0
                    for hh in (0, 1):
                        gt = gts[hh]
                        for c in range(int(C[w, hh])):
                            col = int(base[w, hh]) + c
                            st = spool.tile([P, P], f32, tag="S")
                            nc.vector.tensor_scalar(
                                out=st[:], in0=iota_t[:],
                                scalar1=d_t[:, col:col + 1],
                                scalar2=n_t[:, col:col + 1],
                                op0=Alu.is_equal, op1=Alu.mult)
                            nc.tensor.matmul(
                                out=aggT[:], lhsT=gt[:, c, :], rhs=st[:],
                                start=(cc == 0), stop=(cc == tot - 1))
                            cc += 1
                    agg_s = wpool.tile([P, P], f32, tag="aggs")
                    nc.scalar.copy(out=agg_s[:], in_=aggT[:])
                    outp = psB.tile([P, H], f32, space="PSUM", tag="out")
                    nc.scalar.copy(out=outp[:], in_=bias_t[:])
                    nc.tensor.matmul(out=outp[:], lhsT=agg_s[:], rhs=W_t[:],
                                     start=False, stop=True,
                                     skip_group_check=True)
                    hw_s = wpool.tile([P, H], f32, tag="hw")
                    if relu:
                        nc.scalar.activation(out=hw_s[:], in_=outp[:], func=Act.Relu)
                    else:
                        nc.scalar.copy(out=hw_s[:], in_=outp[:])
                    out_cb(w, hw_s)

            # ---- conv1 ----
            def c1_out(w, hw_s):
                nc.sync.dma_start(out=h1_shard[w * P:(w + 1) * P, :], in_=hw_s[:])

            conv(C1, pp["base1"], i1_t, d1_t, n1_t, xlo, xhi, W1_t, b1_t,
                 True, CMAX1, c1_out)

            # ---- exchange h1 ----
            nc.gpsimd.collective_compute(
                "AllGather", mybir.AluOpType.bypass,
                replica_groups=[list(range(CORES))],
                ins=[h1_shard[:].opt()], outs=[h1_full[:].opt()])

            # ---- conv2 + pooling ----
            gpl = psP.tile([GPC, H], f32, space="PSUM", tag="gpl")

            def c2_out(w, hw_s):
                sp = spool.tile([P, GPC], f32, tag="SP")
                nc.vector.tensor_scalar(
                    out=sp[:], in0=io16_t[:],
                    scalar1=bl_t[:, w:w + 1], scalar2=ic_t[:, w:w + 1],
                    op0=Alu.is_equal, op1=Alu.mult)
                nc.tensor.matmul(out=gpl[:], lhsT=sp[:], rhs=hw_s[:],
                                 start=(w == 0), stop=(w == WPC - 1),
                                 skip_group_check=True)

            conv(C2, pp["base2"], i2_t, d2_t, n2_t,
                 h1_full[:T2, :], h1_full[T2:, :], W2_t, b2_t,
                 False, CMAX2, c2_out)

            # ---- head ----
            l1w_t = load_const(l1w, l1w)
            l2w_t = load_const(l2w, l2w)
            l1b_t = load_const(l1b, l1b)
            l2b_t = load_const(l2b, l2b)
            gam_t = load_const(gamT, gamT)
            bet_t = load_const(betT, betT)
            id16_t = load_const(id16, id16)

            g_s = wpool.tile([GPC, H], f32, tag="g")
            nc.scalar.copy(out=g_s[:], in_=gpl[:])
            # transpose g -> [H, GPC]
            gT_p = psH.tile([H, GPC], f32, space="PSUM", tag="head")
            nc.tensor.transpose(out=gT_p[:], in_=g_s[:], identity=id16_t[:])
            gT_s = wpool.tile([H, GPC], f32, tag="gTs")
            nc.scalar.copy(out=gT_s[:], in_=gT_p[:])
            # z1 = relu(g @ l1w + l1b)
            z1_p = psH.tile([GPC, H], f32, space="PSUM", tag="head")
            nc.scalar.copy(out=z1_p[:], in_=l1b_t[:])
            nc.tensor.matmul(out=z1_p[:], lhsT=gT_s[:], rhs=l1w_t[:],
                             start=False, stop=True, skip_group_check=True)
            z1_s = wpool.tile([GPC, H], f32, tag="z1s")
            nc.scalar.activation(out=z1_s[:], in_=z1_p[:], func=Act.Relu)
            # layernorm over H
            ssum = wpool.tile([GPC, 1], f32, tag="ssum")
            nc.vector.tensor_reduce(out=ssum[:], in_=z1_s[:],
                                    axis=mybir.AxisListType.X, op=Alu.add)
            sq = wpool.tile([GPC, H], f32, tag="sq")
            sqs = wpool.tile([GPC, 1], f32, tag="sqs")
            nc.scalar.activation(out=sq[:], in_=z1_s[:], func=Act.Square,
                                 accum_out=sqs[:])
            mu = wpool.tile([GPC, 1], f32, tag="mu")
            nc.vector.tensor_scalar_mul(out=mu[:], in0=ssum[:], scalar1=1.0 / H)
            m2 = wpool.tile([GPC, 1], f32, tag="m2")
            nc.vector.tensor_scalar_mul(out=m2[:], in0=sqs[:], scalar1=1.0 / H)
            musq = wpool.tile([GPC, 1], f32, tag="musq")
            nc.vector.tensor_mul(out=musq[:], in0=mu[:], in1=mu[:])
            var = wpool.tile([GPC, 1], f32, tag="var")
            nc.vector.tensor_sub(out=var[:], in0=m2[:], in1=musq[:])
            nc.vector.tensor_scalar_add(out=var[:], in0=var[:], scalar1=1e-5)
            sd = wpool.tile([GPC, 1], f32, tag="sd")
            nc.scalar.sqrt(out=sd[:], in_=var[:])
            rstd = wpool.tile([GPC, 1], f32, tag="rstd")
            nc.vector.reciprocal(out=rstd[:], in_=sd[:])
            zc = wpool.tile([GPC, H], f32, tag="zc")
            nc.vector.tensor_scalar(out=zc[:], in0=z1_s[:], scalar1=mu[:, :1],
                                    scalar2=rstd[:, :1],
                                    op0=Alu.subtract, op1=Alu.mult)
            zg = wpool.tile([GPC, H], f32, tag="zg")
            nc.vector.tensor_mul(out=zg[:], in0=zc[:], in1=gam_t[:])
            nc.vector.tensor_add(out=zg[:], in0=zg[:], in1=bet_t[:])
            # logits = z @ l2w + l2b
            zT_p = psH.tile([H, GPC], f32, space="PSUM", tag="head")
            nc.tensor.transpose(out=zT_p[:], in_=zg[:], identity=id16_t[:])
            zT_s = wpool.tile([H, GPC], f32, tag="zTs")
            nc.scalar.copy(out=zT_s[:], in_=zT_p[:])
            lo_p = psH.tile([GPC, CO], f32, space="PSUM", tag="head")
            nc.scalar.copy(out=lo_p[:], in_=l2b_t[:])
            nc.tensor.matmul(out=lo_p[:], lhsT=zT_s[:], rhs=l2w_t[:],
                             start=False, stop=True, skip_group_check=True)
            lo_s = wpool.tile([GPC, CO], f32, tag="los")
            nc.scalar.copy(out=lo_s[:], in_=lo_p[:])
            nc.sync.dma_start(out=logitsT[:], in_=lo_s[:])

    nc.compile()
    return nc


def _make_in_maps(pp, W1, b1, W2, b2, lin1_W, lin1_b, ln_gamma, ln_beta,
                  lin2_W, lin2_b):
    x = pp["x"]
    iota_np = np.tile(np.arange(P, dtype=np.float32)[None, :], (P, 1))
    io16_np = np.tile(np.arange(GPC, dtype=np.float32)[None, :], (P, 1))
    id16_np = np.eye(GPC, dtype=np.float32)
    consts = {
        "xlo": np.ascontiguousarray(x[:T1]),
        "xhi": np.ascontiguousarray(x[T1:]),
        "W1": np.asarray(W1, np.float32),
        "W2": np.asarray(W2, np.float32),
        "l1w": np.asarray(lin1_W, np.float32),
        "l2w": np.asarray(lin2_W, np.float32),
        "b1b": np.tile(np.asarray(b1, np.float32)[None, :], (P, 1)),
        "b2b": np.tile(np.asarray(b2, np.float32)[None, :], (P, 1)),
        "l1b": np.tile(np.asarray(lin1_b, np.float32)[None, :], (GPC, 1)),
        "l2b": np.tile(np.asarray(lin2_b, np.float32)[None, :], (GPC, 1)),
        "gam": np.tile(np.asarray(ln_gamma, np.float32)[None, :], (GPC, 1)),
        "bet": np.tile(np.asarray(ln_beta, np.float32)[None, :], (GPC, 1)),
        "iota": iota_np,
        "iot16": io16_np,
        "id16": id16_np,
    }
    in_maps = []
    for k in range(CORES):
        m = dict(consts)
        m["i1"] = pp["i1s"][k]
        m["d1"] = pp["d1s"][k]
        m["n1"] = pp["n1s"][k]
        m["i2"] = pp["i2s"][k]
        m["d2"] = pp["d2s"][k]
        m["n2"] = pp["n2s"][k]
        m["bl"] = pp["bls"][k]
        m["ic"] = pp["ics"][k]
        in_maps.append(m)
    return in_maps


def kernel(x, edge_index, edge_weight, batch_idx,
           W1, b1, W2, b2, lin1_W, lin1_b, ln_gamma, ln_beta,
           lin2_W, lin2_b, _trace=False, _nc_cache={}):
    from concourse.bass_utils import run_bass_kernel_spmd

    pp = _host_prep(x, edge_index, edge_weight, batch_idx)

    key = (pp["WPC"], pp["TC1"], pp["TC2"],
           tuple(pp["C1"].reshape(-1)), tuple(pp["C2"].reshape(-1)))
    nc = _nc_cache.get(key)
    if nc is None:
        nc = _build_program(pp)
        _nc_cache[key] = nc

    in_maps = _make_in_maps(pp, W1, b1, W2, b2, lin1_W, lin1_b,
                            ln_gamma, ln_beta, lin2_W, lin2_b)
    res = run_bass_kernel_spmd(nc, in_maps, core_ids=list(range(CORES)),
                               trace=_trace)
    out = np.concatenate([res.results[k]["logits"] for k in range(CORES)],
                         axis=0).astype(np.float32)
    kernel._last_exec_time_ns = res.exec_time_ns
    return out
